# revision 13
# baseline (speedup 1.0000x reference)
"""Trainium2 Bass kernel for nn_DBI_10187662426595 (moe_routing).

Strategy: data-parallel over batch B=32 -> 4 batch elements per core on 8
NeuronCores.

  - feats are DMA'd in their natural [t, d] row layout (contiguous 4KB rows)
    and transposed on the TensorEngine into [d_in, (b, t)] conv operands;
    conv weights are pre-transposed on the host to [d_in, (tap, d_out)] so
    they stream as contiguous rows straight into the matmul lhsT slot.
  - all matmuls run in float32r (full PE rate at free-dim >= 256, ~2^-12
    rel precision); SWDGE cast-DMA / PSUM-copy producers do the rounding.
  - the k=5 stride-4 down-conv reads its rhs through a stride-4 access
    pattern over a zero-padded SBUF tile; the slow->fast path does conv
    first (4x less PE work), then linear interpolation as 4 phase-wise DVE
    ops (exactly matching align_corners=False), then transposes the deltas
    back to row layout.
  - blends run in row layout: out = (delta * w[b]) + residual as a single
    fused scalar_tensor_tensor per delta, with per-batch gate weights read
    as per-partition scalar APs.  Stores are contiguous row DMAs.
  - scores: per-core MLP over its own 4 batch elements for all 8 used
    pairs; the only cross-core exchange is a 32-byte AllReduce of per-pair
    score sums (the score.mean() >= 0.3 gates).  The gate chain folds the
    reference's sequential `where` priority into per-pair weights.
"""
import numpy as np
from contextlib import ExitStack

import concourse.mybir as mybir
import concourse.tile as tile
import concourse.bacc as bacc
from concourse.bass_utils import run_bass_kernel_spmd

F32 = mybir.dt.float32
F32R = mybir.dt.float32r
BF16 = mybir.dt.bfloat16
AX = mybir.AxisListType
OP = mybir.AluOpType
AF = mybir.ActivationFunctionType

N_CORES = 8
TF, TS, B, BL, D, H = 512, 128, 32, 4, 1024, 256
DB = D // 128               # 8 d-blocks of 128 partitions
TC = TF // 128              # 4 t-chunks per fast-feat batch row
THRESH = 0.3

PAIRS = [(0, 2), (0, 3), (1, 2), (1, 3), (2, 0), (2, 1), (3, 0), (3, 1)]
P_FLAT = [i * 4 + j for (i, j) in PAIRS]
F2S_OF_PAIR = {(0, 2): 0, (0, 3): 1, (1, 2): 2, (1, 3): 3}
SEC_PRI = [(0, 2), (1, 3), (4, 6), (5, 7)]
INTERP = [(0.375, 0.625, 0), (0.125, 0.875, 0),
          (0.875, 0.125, 1), (0.625, 0.375, 1)]
TP = TF + 4                 # padded fast time axis (2 zeros each side)


def _emit(nc, tc, ctx):
    d = {k: nc.dram_tensor(k, shp, F32, kind="ExternalInput").ap()
         for k, shp in [
             ("f0", [TF, BL, D]), ("f1", [TF, BL, D]),
             ("f2", [TS, BL, D]), ("f3", [TS, BL, D]),
             ("s2fT", [D, D]), ("s2fb", [1, D]),
             ("f2sT", [4, D, 5 * D]),       # [cv, d_in, (tap, d_out)]
             ("w1a", [8, 2 * D, H]), ("b1a", [8, H]),
             ("w2a", [8, H]), ("b2row", [1, 8]),
             ("ones128", [1, 128]), ("eye4", [4, 4]),
             ("eye128", [128, 128]),
         ]}
    o = {k: nc.dram_tensor(k, shp, F32, kind="ExternalOutput").ap()
         for k, shp in [
             ("o0", [TF, BL, D]), ("o1", [TF, BL, D]),
             ("o2", [TS, BL, D]), ("o3", [TS, BL, D]),
         ]}

    const = ctx.enter_context(tc.tile_pool(name="const", bufs=1))
    gpool = ctx.enter_context(tc.tile_pool(name="g", bufs=1))
    ups_p = ctx.enter_context(tc.tile_pool(name="ups", bufs=1))
    dram = ctx.enter_context(tc.tile_pool(name="dram", bufs=1, space="DRAM"))

    ones_r = const.tile([1, 128], BF16, tag="ones_r")
    nc.gpsimd.dma_start(out=ones_r[:], in_=d["ones128"][:])
    eye4_r = const.tile([4, 4], BF16, tag="eye4")
    nc.gpsimd.dma_start(out=eye4_r[:], in_=d["eye4"][:])
    eye_f = const.tile([128, 128], F32, tag="eye_f")
    nc.sync.dma_start(out=eye_f[:], in_=d["eye128"][:])
    eye_b = const.tile([128, 128], BF16, tag="eye_b")
    nc.gpsimd.dma_start(out=eye_b[:], in_=d["eye128"][:])
    b2row_r = const.tile([1, 8], BF16, tag="b2row")
    nc.gpsimd.dma_start(out=b2row_r[:], in_=d["b2row"][:])
    s2fb_sb = const.tile([128, DB], F32, tag="s2fb")
    nc.sync.dma_start(out=s2fb_sb[:],
                      in_=d["s2fb"].rearrange("o (k p) -> p (o k)", p=128))
    zero8 = const.tile([128, 8], F32, tag="zero8")
    nc.vector.memset(zero8[:], 0.0)

    g_raw = [gpool.tile([128, DB * BL], F32, tag=f"graw{i}",
                        name=f"graw{i}") for i in range(4)]
    g_f = [gpool.tile([128, DB * BL], BF16, tag=f"gf{i}",
                      name=f"gf{i}") for i in range(4)]

    # slow->fast deltas in ROW layout: one [t=128, D] bf16 tile per
    # (feat, local batch, fast t-chunk)
    ups = {(fi, b, tci): ups_p.tile([128, D], BF16, tag=f"up{fi}_{b}_{tci}",
                                    name=f"up{fi}_{b}_{tci}")
           for fi in (2, 3) for b in range(BL) for tci in range(TC)}

    rows_p = ctx.enter_context(tc.tile_pool(name="rows", bufs=2))

    def load_transposed(feat_key, kb_tiles, t_len, ps_pool, pad):
        """DMA feat rows, PE-transpose into [d, (b, t)] tiles.

        kb_tiles: list of 8 destination APs [128, BL*(t_len+pad*4)] (f32r).
        """
        ntc = t_len // 128
        for b in range(BL):
            for tci in range(ntc):
                rw = rows_p.tile([128, D], F32, tag="rw")
                nc.sync.dma_start(
                    out=rw[:],
                    in_=d[feat_key][tci * 128:(tci + 1) * 128, b, :])
                for kb in range(DB):
                    pst = ps_pool.tile([128, 128], F32, tag="pst",
                                       name="pst")
                    nc.tensor.transpose(pst[:],
                                        rw[:, kb * 128:(kb + 1) * 128],
                                        eye_f[:])
                    off = b * (t_len + 4 * pad) + 2 * pad + tci * 128
                    nc.scalar.copy(kb_tiles[kb][:, off:off + 128], pst[:])

    # ---------------- phase 0: slow feats, up path ----------------
    with ExitStack() as ph0:
        x23_p = ph0.enter_context(tc.tile_pool(name="x23", bufs=1))
        s2f_p = ph0.enter_context(tc.tile_pool(name="s2f", bufs=1))
        yp_p = ph0.enter_context(tc.tile_pool(name="yp", bufs=2))
        it_p = ph0.enter_context(tc.tile_pool(name="itmp", bufs=2))
        upc_p = ph0.enter_context(tc.tile_pool(name="upc", bufs=2))

        x23 = {}
        for fi in (2, 3):
            tiles = [x23_p.tile([128, BL * TS], F32R, tag=f"x{fi}_{kb}",
                                name=f"x{fi}_{kb}") for kb in range(DB)]
            with ExitStack() as tps:
                ps_t = tps.enter_context(
                    tc.tile_pool(name="ps_t", bufs=2, space="PSUM"))
                load_transposed(f"f{fi}", [t[:] for t in tiles], TS, ps_t, 0)
            for kb in range(DB):
                x23[(fi, kb)] = tiles[kb]
                nc.vector.tensor_reduce(
                    g_raw[fi][:].rearrange("p (k b) -> p k b", k=DB)[:, kb, :],
                    tiles[kb][:].bitcast(F32).rearrange(
                        "p (b t) -> p b t", b=BL),
                    AX.X, OP.add)
            nc.vector.tensor_scalar_mul(g_f[fi][:], g_raw[fi][:], 1.0 / TS)

        s2f_sb = []
        for kb in range(DB):
            t_ = s2f_p.tile([128, D], F32R, tag=f"s2f{kb}")
            nc.gpsimd.dma_start(out=t_[:],
                                in_=d["s2fT"][kb * 128:(kb + 1) * 128, :])
            s2f_sb.append(t_)

        for fi in (2, 3):
            upcs = []
            with ExitStack() as ups_s:
                ps_up = ups_s.enter_context(
                    tc.tile_pool(name="ps_up", bufs=1, space="PSUM"))
                psy = [ps_up.tile([128, BL * TS], F32, tag=f"psy{dout}",
                                  name=f"psy{dout}")
                       for dout in range(DB)]
                for kb in range(DB):
                    for dout in range(DB):
                        nc.tensor.matmul(
                            psy[dout][:],
                            s2f_sb[kb][:, dout * 128:(dout + 1) * 128],
                            x23[(fi, kb)][:],
                            start=(kb == 0), stop=(kb == DB - 1))
                for dout in range(DB):
                    yp = yp_p.tile([128, BL * (TS + 2)], F32, tag="yp")
                    ypv = yp[:].rearrange("p (b t) -> p b t", b=BL)
                    nc.scalar.activation(
                        ypv[:, :, 1:TS + 1],
                        psy[dout][:].rearrange("p (b t) -> p b t", b=BL),
                        AF.Identity, bias=s2fb_sb[:, dout:dout + 1])
                    nc.vector.tensor_copy(ypv[:, :, 0:1], ypv[:, :, 1:2])
                    nc.vector.tensor_copy(ypv[:, :, TS + 1:TS + 2],
                                          ypv[:, :, TS:TS + 1])
                    # interp -> up columns [d=128, (b, 512)] bf16
                    upc = upc_p.tile([128, BL * TF], BF16, tag="upc",
                                     bufs=8)
                    upv = upc[:].rearrange("p (b u r) -> p r b u", r=4, b=BL)
                    for r, (ca, cb, s0) in enumerate(INTERP):
                        tmp = it_p.tile([128, BL * TS], F32, tag="it")
                        tv = tmp[:].rearrange("p (b t) -> p b t", b=BL)
                        nc.vector.tensor_scalar_mul(
                            tv[:, :, :], ypv[:, :, s0 + 1:s0 + 1 + TS], cb)
                        nc.vector.scalar_tensor_tensor(
                            upv[:, r, :, :], ypv[:, :, s0:s0 + TS], ca,
                            tv[:, :, :], OP.mult, OP.add)
                    upcs.append(upc)
            # transpose up columns back to row layout (conv PSUM now free)
            with ExitStack() as tps:
                ps_t2 = tps.enter_context(
                    tc.tile_pool(name="ps_t2", bufs=2, space="PSUM"))
                for dout in range(DB):
                    ucv = upcs[dout][:].rearrange("p (b t) -> p b t", b=BL)
                    for b in range(BL):
                        for tci in range(TC):
                            pst2 = ps_t2.tile([128, 128], BF16,
                                              tag="pst2", name="pst2")
                            nc.tensor.transpose(
                                pst2[:],
                                ucv[:, b, tci * 128:(tci + 1) * 128],
                                eye_b[:])
                            nc.scalar.copy(
                                ups[(fi, b, tci)]
                                [:, dout * 128:(dout + 1) * 128],
                                pst2[:])

    # ------------- phases A / B: fast feats, down convs, MLP -------------
    xf_p = ctx.enter_context(tc.tile_pool(name="xf", bufs=1))
    out_p = ctx.enter_context(tc.tile_pool(name="out", bufs=2))
    rr_p = ctx.enter_context(tc.tile_pool(name="rr", bufs=2))
    dkr_p = ctx.enter_context(tc.tile_pool(name="dkr", bufs=1))
    dc_p = ctx.enter_context(tc.tile_pool(name="dc", bufs=2))

    def load_fast_feat(fj):
        tiles = [xf_p.tile([128, BL * TP], F32R, tag=f"xf{kb}",
                           name=f"xf{kb}") for kb in range(DB)]
        for kb in range(DB):
            tv = tiles[kb][:].rearrange("p (b t) -> p b t", b=BL)
            nc.vector.tensor_copy(
                tv[:, :, 0:2], zero8[:].rearrange("p (b t) -> p b t", b=BL))
            nc.vector.tensor_copy(
                tv[:, :, TF + 2:TF + 4],
                zero8[:].rearrange("p (b t) -> p b t", b=BL))
        with ExitStack() as tps:
            ps_tf = tps.enter_context(
                tc.tile_pool(name="ps_tf", bufs=2, space="PSUM"))
            load_transposed(f"f{fj}", [t[:] for t in tiles], TF, ps_tf, 1)
        for kb in range(DB):
            nc.vector.tensor_reduce(
                g_raw[fj][:].rearrange("p (k b) -> p k b", k=DB)[:, kb, :],
                tiles[kb][:].bitcast(F32).rearrange(
                    "p (b t) -> p b t", b=BL),
                AX.X, OP.add)
        nc.vector.tensor_scalar_mul(g_f[fj][:], g_raw[fj][:], 1.0 / TF)
        return tiles

    with ExitStack() as convs:
        w_p = convs.enter_context(tc.tile_pool(name="wst", bufs=2))

        def down_conv_to_rows(cv, xf, keep_rows):
            """Full k=5 stride-4 conv; returns per-b row tiles [128, D]."""
            with ExitStack() as cs:
                ps_c = cs.enter_context(
                    tc.tile_pool(name="ps_c", bufs=1, space="PSUM"))
                psd = [ps_c.tile([128, BL * TS], F32, tag=f"psd{dout}",
                                 name=f"psd{dout}") for dout in range(DB)]
                # host weight layout: [cv, d_in, (tap, d_out)]; stream
                # slices of <= 2 taps per d_in block
                for kb in range(DB):
                    for (t0, ntap) in ((0, 2), (2, 2), (4, 1)):
                        wt = w_p.tile([128, 2 * D], F32R, tag="wt")
                        nc.gpsimd.dma_start(
                            out=wt[:, 0:ntap * D],
                            in_=d["f2sT"][cv, kb * 128:(kb + 1) * 128,
                                          t0 * D:(t0 + ntap) * D])
                        for tl in range(ntap):
                            tap = t0 + tl
                            rhs = xf[kb][:].rearrange(
                                "p (b t) -> p b t", b=BL)[
                                :, :, tap:tap + 512].rearrange(
                                "p b (t s) -> p b t s", s=4)[:, :, :, 0]
                            for dout in range(DB):
                                nc.tensor.matmul(
                                    psd[dout][:].rearrange(
                                        "p (b t) -> p b t", b=BL),
                                    wt[:, tl * D + dout * 128:
                                       tl * D + (dout + 1) * 128],
                                    rhs,
                                    start=(kb == 0 and tap == 0),
                                    stop=(kb == DB - 1 and tap == 4))
                # park as bf16 columns, then transpose to row layout
                dcols = []
                for dout in range(DB):
                    dcol = dc_p.tile([128, BL * TS], BF16, tag="dc", bufs=8)
                    nc.scalar.copy(dcol[:], psd[dout][:])
                    dcols.append(dcol)
            rows = []
            with ExitStack() as tps:
                ps_td = tps.enter_context(
                    tc.tile_pool(name="ps_td", bufs=2, space="PSUM"))
                for b in range(BL):
                    if keep_rows:
                        drow = dkr_p.tile([128, D], BF16,
                                          tag=f"dk{cv}_{b}",
                                          name=f"dk{cv}_{b}")
                    else:
                        drow = dc_p.tile([128, D], BF16, tag="drow", bufs=4)
                    for dout in range(DB):
                        pstd = ps_td.tile([128, 128], BF16, tag="pstd",
                                          name="pstd")
                        nc.tensor.transpose(
                            pstd[:],
                            dcols[dout][:, b * TS:(b + 1) * TS],
                            eye_b[:])
                        nc.scalar.copy(drow[:, dout * 128:(dout + 1) * 128],
                                       pstd[:])
                    rows.append(drow)
            return rows

        # --- phase A: f1 -> convs (1,2), (1,3); row deltas parked ---
        xf1 = load_fast_feat(1)
        dk = {}
        for cv in (2, 3):
            dk[cv] = down_conv_to_rows(cv, xf1, keep_rows=True)

        # --- phase B: f0 ---
        xf0 = load_fast_feat(0)

        # MLP for all 8 pairs on this core's 4 batch elements
        sc_row = const.tile([1, 32], F32, tag="sc_row")
        with ExitStack() as mlps:
            mlp_w = mlps.enter_context(tc.tile_pool(name="mlpw", bufs=2))
            mlp_s = mlps.enter_context(tc.tile_pool(name="mlps", bufs=1))
            ps_m = mlps.enter_context(
                tc.tile_pool(name="ps_m", bufs=1, space="PSUM"))
            for a, (pi, pj) in enumerate(PAIRS):
                ph0_ = ps_m.tile([128, BL], F32, tag="h0")
                ph1_ = ps_m.tile([128, BL], F32, tag="h1")
                for kbk in range(16):
                    wt1 = mlp_w.tile([128, H], BF16, tag="wt1")
                    nc.gpsimd.dma_start(
                        out=wt1[:],
                        in_=d["w1a"][a, kbk * 128:(kbk + 1) * 128, :])
                    fi = pi if kbk < 8 else pj
                    col = kbk % 8
                    rhs = g_f[fi][:].rearrange(
                        "p (k b) -> p k b", k=DB)[:, col, :]
                    nc.tensor.matmul(ph0_[:], wt1[:, 0:128], rhs,
                                     start=(kbk == 0), stop=(kbk == 15))
                    nc.tensor.matmul(ph1_[:], wt1[:, 128:256], rhs,
                                     start=(kbk == 0), stop=(kbk == 15))
                b1_sb = mlp_s.tile([128, 2], F32, tag="b1")
                nc.sync.dma_start(
                    out=b1_sb[:],
                    in_=d["b1a"][a:a + 1, :].rearrange(
                        "o (k p) -> p (o k)", p=128))
                h0 = mlp_s.tile([128, BL], BF16, tag="h0s")
                h1 = mlp_s.tile([128, BL], BF16, tag="h1s")
                nc.scalar.activation(h0[:], ph0_[:], AF.Relu,
                                     bias=b1_sb[:, 0:1])
                nc.scalar.activation(h1[:], ph1_[:], AF.Relu,
                                     bias=b1_sb[:, 1:2])
                w2_sb = mlp_s.tile([128, 2], BF16, tag="w2")
                nc.gpsimd.dma_start(
                    out=w2_sb[:],
                    in_=d["w2a"][a:a + 1, :].rearrange(
                        "o (k p) -> p (o k)", p=128))
                ps_s = ps_m.tile([BL, 1], F32, tag="ss")
                nc.tensor.matmul(ps_s[:], h0[:], w2_sb[:, 0:1],
                                 start=True, stop=False)
                nc.tensor.matmul(ps_s[:], h1[:], w2_sb[:, 1:2],
                                 start=False, stop=False)
                nc.tensor.matmul(ps_s[:], ones_r[0:1, 0:BL],
                                 b2row_r[:, a:a + 1], start=False, stop=True)
                sc_col = mlp_s.tile([BL, 1], BF16, tag="scc")
                nc.scalar.activation(sc_col[:], ps_s[:], AF.Sigmoid)
                ps_r = ps_m.tile([1, BL], F32, tag="sr")
                nc.tensor.matmul(ps_r[:], sc_col[:], eye4_r[:],
                                 start=True, stop=True)
                nc.vector.tensor_copy(sc_row[:, 4 * a:4 * a + 4], ps_r[:])

        # gates: AllReduce per-pair score sums across all 32 batch elements
        sums = const.tile([1, 8], F32, tag="sums")
        nc.vector.tensor_reduce(sums[:],
                                sc_row[:].rearrange("o (a b) -> o a b", a=8),
                                AX.X, OP.add)
        ar_in = dram.tile([1, 8], F32)
        ar_out = dram.tile([1, 8], F32, addr_space="Shared")
        nc.sync.dma_start(out=ar_in[:], in_=sums[:])
        nc.gpsimd.collective_compute(
            "AllReduce", OP.add, ins=[ar_in[:]], outs=[ar_out[:]],
            replica_groups=[list(range(N_CORES))])
        tot = const.tile([1, 8], F32, tag="tot")
        nc.sync.dma_start(out=tot[:], in_=ar_out[:])

        cond = const.tile([1, 8], F32, tag="cond")
        nc.vector.tensor_scalar(cond[:], tot[:], 1.0 / B, THRESH,
                                OP.mult, OP.is_ge)
        adj = const.tile([1, 8], F32, tag="adj")
        nc.vector.memset(adj[:], 1.0)
        for sec, pri in SEC_PRI:
            nc.vector.tensor_scalar(adj[:, sec:sec + 1],
                                    cond[:, pri:pri + 1], -1.0, 1.0,
                                    OP.mult, OP.add)
        eff = const.tile([1, 8], F32, tag="eff")
        nc.vector.tensor_tensor(eff[:], cond[:], adj[:], OP.mult)
        w_row = const.tile([1, 32], F32, tag="w_row")
        for a in range(8):
            nc.vector.tensor_scalar(w_row[:, 4 * a:4 * a + 4],
                                    sc_row[:, 4 * a:4 * a + 4],
                                    eff[:, a:a + 1], None, OP.mult)
        w_all = const.tile([128, 32], F32, tag="w_all")
        nc.gpsimd.partition_broadcast(w_all[:], w_row[:])

        def wap(a, b):
            return w_all[:, 4 * a + b:4 * a + b + 1]

        # --- convs (0,2), (0,3) + out2/out3 blends (row layout) ---
        for jj, cv in ((2, 0), (3, 1)):
            rows0 = down_conv_to_rows(cv, xf0, keep_rows=False)
            a0 = PAIRS.index((0, jj))
            a1 = PAIRS.index((1, jj))
            cv1 = F2S_OF_PAIR[(1, jj)]
            for b in range(BL):
                fr = rr_p.tile([128, D], F32, tag="rr")
                nc.sync.dma_start(out=fr[:], in_=d[f"f{jj}"][:, b, :])
                s1 = out_p.tile([128, D], F32, tag="out")
                nc.vector.scalar_tensor_tensor(
                    s1[:], rows0[b][:], wap(a0, b), fr[:],
                    OP.mult, OP.add)
                ot = out_p.tile([128, D], F32, tag="out")
                nc.vector.scalar_tensor_tensor(
                    ot[:], dk[cv1][b][:], wap(a1, b), s1[:],
                    OP.mult, OP.add)
                nc.sync.dma_start(out=o[f"o{jj}"][:, b, :], in_=ot[:])

        # --- out0 / out1 blends (row layout, re-read residual rows) ---
        for j in (0, 1):
            a2 = PAIRS.index((2, j))
            a3 = PAIRS.index((3, j))
            for b in range(BL):
                for tci in range(TC):
                    fr = rr_p.tile([128, D], F32, tag="rr")
                    nc.sync.dma_start(
                        out=fr[:],
                        in_=d[f"f{j}"][tci * 128:(tci + 1) * 128, b, :])
                    s1 = out_p.tile([128, D], F32, tag="out")
                    nc.vector.scalar_tensor_tensor(
                        s1[:], ups[(2, b, tci)][:], wap(a2, b), fr[:],
                        OP.mult, OP.add)
                    ot = out_p.tile([128, D], F32, tag="out")
                    nc.vector.scalar_tensor_tensor(
                        ot[:], ups[(3, b, tci)][:], wap(a3, b), s1[:],
                        OP.mult, OP.add)
                    nc.sync.dma_start(
                        out=o[f"o{j}"][tci * 128:(tci + 1) * 128, b, :],
                        in_=ot[:])


_PROGRAM = None


def build_program():
    global _PROGRAM
    if _PROGRAM is None:
        nc = bacc.Bacc(None, target_bir_lowering=False, debug=False,
                       num_devices=N_CORES)
        with tile.TileContext(nc) as tc:
            with ExitStack() as ctx:
                _emit(nc, tc, ctx)
        nc.compile()
        _PROGRAM = nc
    return _PROGRAM


def _host_prep(inputs):
    # f2s_w [cv, d_out, d_in, tap] -> [cv, d_in, tap, d_out] -> flat last two
    f2sT = np.ascontiguousarray(
        np.asarray(inputs["f2s_w"]).transpose(0, 2, 3, 1)
    ).reshape(4, D, 5 * D).astype(np.float32)
    s2fT = np.ascontiguousarray(
        np.asarray(inputs["s2f_w"])[:, :, 0].T).astype(np.float32)
    w1a = np.ascontiguousarray(
        np.asarray(inputs["mlp_w1"])[P_FLAT]).astype(np.float32)
    b1a = np.ascontiguousarray(
        np.asarray(inputs["mlp_b1"])[P_FLAT]).astype(np.float32)
    w2a = np.ascontiguousarray(
        np.asarray(inputs["mlp_w2"])[P_FLAT, :, 0]).astype(np.float32)
    b2row = np.ascontiguousarray(
        np.asarray(inputs["mlp_b2"])[P_FLAT, 0][None, :]).astype(np.float32)
    s2fb = np.ascontiguousarray(
        np.asarray(inputs["s2f_b"])[None, :]).astype(np.float32)
    common = {
        "f2sT": f2sT, "s2fT": s2fT, "s2fb": s2fb,
        "w1a": w1a, "b1a": b1a, "w2a": w2a, "b2row": b2row,
        "ones128": np.ones((1, 128), np.float32),
        "eye4": np.eye(4, dtype=np.float32),
        "eye128": np.eye(128, dtype=np.float32),
    }
    in_maps = []
    for c in range(N_CORES):
        m = dict(common)
        for i in range(4):
            m[f"f{i}"] = np.ascontiguousarray(
                np.asarray(inputs[f"feat{i}"])[:, 4 * c:4 * c + 4, :]
            ).astype(np.float32)
        in_maps.append(m)
    return in_maps


def kernel(**inputs):
    nc = build_program()
    in_maps = _host_prep(inputs)
    res = run_bass_kernel_spmd(nc, in_maps, core_ids=list(range(N_CORES)),
                               trace=False)
    outs = []
    for j, t_len in ((0, TF), (1, TF), (2, TS), (3, TS)):
        full = np.empty((t_len, B, D), np.float32)
        for c in range(N_CORES):
            full[:, 4 * c:4 * c + 4, :] = res.results[c][f"o{j}"]
        outs.append(full)
    return tuple(outs)
